# revision 15
# baseline (speedup 1.0000x reference)
"""Multi-head attention (B=2, S=2048, D=1024, H=16) on 8 TRN2 NeuronCores.

Sharding: batch x head-group. Core c handles batch b=c//4 and heads
[4g, 4g+4) with g=c%4 (column-parallel QKV projections, row-parallel
output projection). Each core emits a partial [S, D] output; the host
sums the 4 partials per batch and adds bf (the row-parallel reduce).

v2 schedule: the attention inner loop is ScalarE-bound (exp), so all
projection / output-projection matmuls are interleaved into the
attention emission stream in ~0.9us sub-quanta to keep the PE
continuously busy (full HAM clock). Causal masking is a multiplicative
bf16 0/1 tile applied post-exp on GpSimd (SBUF-only engine, otherwise
idle). Softmax renormalization per (head-pair, chunk): row sums fall
out of the AV matmul via a ones column in vh (even local heads put
values in px partitions 0:64 with the ones column at slot 96; odd heads
at 64:128 with ones at slot 32, so normalized outputs land directly in
both halves of xn2 without any partition-crossing DMA),
reciprocal_approx_fast on the sums row, then K=1 ones-vector matmuls
broadcast 1/s across partitions. AV weights are zero-padded to 128
columns so fast-weight-load stays enabled.
"""

import numpy as np
import ml_dtypes

import concourse.bass as bass
import concourse.tile as tile
from concourse import bacc, mybir
from concourse.bass_utils import run_bass_kernel_spmd

B, S, D, H = 2, 2048, 1024, 16
HD = D // H          # 64
HL = H // 4          # 4 heads per core
PL = HL * HD         # 256 local projection dim
KT = D // 128        # 8 contraction blocks
SB = S // 128        # 16 sequence blocks of 128
CH = S // 512        # 4 sequence chunks of 512
F32 = mybir.dt.float32
DT = mybir.dt.bfloat16
NP_DT = ml_dtypes.bfloat16

_cache = {}
last_results = None

ADD = mybir.AluOpType.add
MUL = mybir.AluOpType.mult
EXP = mybir.ActivationFunctionType.Exp

# ones-column slot in vh per local-head parity (= sums partition in px);
# both 32-aligned so the K=1 broadcast matmuls get legal tile_positions
ONE_EVEN = 64
ONE_ODD = 32


def build_program():
    if "nc" in _cache:
        return _cache["nc"]
    nc = bacc.Bacc("TRN2", target_bir_lowering=False, debug=False, num_devices=8)

    qt_d = nc.dram_tensor("qt", [D, S], DT, kind="ExternalInput")
    kt_d = nc.dram_tensor("kt", [D, S], DT, kind="ExternalInput")
    vt_d = nc.dram_tensor("vt", [D, S], DT, kind="ExternalInput")
    wq_d = nc.dram_tensor("wq", [128, KT, PL], DT, kind="ExternalInput")
    wk_d = nc.dram_tensor("wk", [128, KT, PL], DT, kind="ExternalInput")
    wv_d = nc.dram_tensor("wv", [128, KT, PL], DT, kind="ExternalInput")
    wf_d = nc.dram_tensor("wf", [128, 2, D], DT, kind="ExternalInput")
    bq_d = nc.dram_tensor("bq2", [128, 2], F32, kind="ExternalInput")
    bk_d = nc.dram_tensor("bk2", [128, 2], F32, kind="ExternalInput")
    bv_d = nc.dram_tensor("bv1", [1, PL], F32, kind="ExternalInput")
    tri_d = nc.dram_tensor("tri2", [128, 2, 128], DT, kind="ExternalInput")
    scr_d = nc.dram_tensor("scr", [1, 512], DT, kind="ExternalOutput")
    out_d = nc.dram_tensor("out", [S, D], DT, kind="ExternalOutput")

    with tile.TileContext(nc) as tc:
        with (
            tc.tile_pool(name="singles", bufs=1) as singles,
            tc.tile_pool(name="kp", bufs=8) as kp,
            tc.tile_pool(name="qp", bufs=16) as qp,
            tc.tile_pool(name="vp", bufs=16) as vp,
            tc.tile_pool(name="epool", bufs=17) as epool,
            tc.tile_pool(name="nrm", bufs=2) as nrm,
            tc.tile_pool(name="obp", bufs=3) as obp,
            tc.tile_pool(name="psum", bufs=2, space="PSUM") as psum,
        ):
            wq_sb = singles.tile([128, KT, PL], DT)
            wk_sb = singles.tile([128, KT, PL], DT)
            wv_sb = singles.tile([128, KT, PL], DT)
            wf_sb = singles.tile([128, 2, D], DT)
            bq_sb = singles.tile([128, 2], F32)
            bk_sb = singles.tile([128, 2], F32)
            bv_bc = singles.tile([128, PL], F32)
            tri_sb = singles.tile([128, 2, 128], DT)
            sel2_sb = singles.tile([128, 128], DT)
            sums_bf = singles.tile([128, 512], DT)

            khT = singles.tile([128, 2, S], DT)   # [par*64+d, pair, s]
            qhT = singles.tile([128, 2, S], DT)
            # vh: [s within blk, sblk, local h, slot]; even h: vals 0:64,
            # ones@96; odd h: vals 64:128, ones@32; zeros elsewhere (full
            # 128-wide AV weights keep FWL enabled)
            vh = singles.tile([128, SB, HL, 128], DT)
            xn2 = singles.tile([128, 2, S], DT)   # normalized attn out

            nc.vector.memset(sel2_sb, 0.0)
            nc.vector.memset(sel2_sb[ONE_EVEN : ONE_EVEN + 1, 0:64], 1.0)
            nc.vector.memset(sel2_sb[ONE_ODD : ONE_ODD + 1, 64:128], 1.0)
            nc.vector.memset(sums_bf, 0.0)
            for h in range(HL):
                if h % 2 == 0:
                    nc.vector.memset(vh[:, :, h, 64:65], 1.0)
                    nc.vector.memset(vh[:, :, h, 65:128], 0.0)
                else:
                    nc.vector.memset(vh[:, :, h, 0:32], 0.0)
                    nc.vector.memset(vh[:, :, h, 32:33], 1.0)
                    nc.vector.memset(vh[:, :, h, 33:64], 0.0)

            # ---- DMA issues ----
            # sync ring, priority order: K weights+tensor, then Q
            nc.sync.dma_start(wk_sb, wk_d.ap())
            nc.sync.dma_start(bk_sb, bk_d.ap())
            kt_t = []
            for kk in range(KT):
                t = kp.tile([128, S], DT, tag="k", name=f"kt{kk}")
                nc.sync.dma_start(t, kt_d.ap()[128 * kk : 128 * (kk + 1), :])
                kt_t.append(t)
            nc.sync.dma_start(wq_sb, wq_d.ap())
            nc.sync.dma_start(bq_sb, bq_d.ap())
            qt_t = [[None] * KT, [None] * KT]
            for half in range(2):
                for kk in range(KT):
                    t = qp.tile([128, 1024], DT, tag="q", name=f"q{half}_{kk}")
                    nc.sync.dma_start(
                        t,
                        qt_d.ap()[
                            128 * kk : 128 * (kk + 1), 1024 * half : 1024 * (half + 1)
                        ],
                    )
                    qt_t[half][kk] = t

            # gpsimd ring, gated behind khT chunk 0 so it doesn't steal HBM
            # bandwidth from the critical K/Q prefix: dummy store depends on
            # the first khT projection copy
            dummy_gate = [False]
            vt_t = [[None] * KT, [None] * KT]

            def issue_v_dmas():
                nc.gpsimd.dma_start(scr_d.ap(), khT[0:1, 0, 0:512])
                nc.gpsimd.dma_start(wv_sb, wv_d.ap())
                nc.gpsimd.dma_start(bv_bc, bv_d.ap().to_broadcast([128, PL]))
                nc.gpsimd.dma_start(tri_sb, tri_d.ap())
                for half in range(2):
                    for kk in range(KT):
                        t = vp.tile([128, 1024], DT, tag="v", name=f"v{half}_{kk}")
                        nc.gpsimd.dma_start(
                            t,
                            vt_d.ap()[
                                128 * kk : 128 * (kk + 1),
                                1024 * half : 1024 * (half + 1),
                            ],
                        )
                        vt_t[half][kk] = t
                nc.gpsimd.dma_start(wf_sb, wf_d.ap())
                dummy_gate[0] = True

            # ---- PE work quanta (two ~0.9us parts each) ----
            def kproj_parts(pt, ch):
                cell = {}

                def p1():
                    cell["pp"] = psum.tile([128, 512], F32, tag="C", bufs=2, name="pp")
                    for kk in range(4):
                        nc.tensor.matmul(
                            cell["pp"],
                            lhsT=wk_sb[:, kk, 128 * pt : 128 * (pt + 1)],
                            rhs=kt_t[kk][:, 512 * ch : 512 * (ch + 1)],
                            start=(kk == 0),
                            stop=False,
                        )

                def p2():
                    for kk in range(4, KT):
                        nc.tensor.matmul(
                            cell["pp"],
                            lhsT=wk_sb[:, kk, 128 * pt : 128 * (pt + 1)],
                            rhs=kt_t[kk][:, 512 * ch : 512 * (ch + 1)],
                            start=False,
                            stop=(kk == KT - 1),
                        )
                    nc.vector.tensor_scalar_add(
                        khT[:, pt, 512 * ch : 512 * (ch + 1)],
                        cell["pp"],
                        bk_sb[:, pt : pt + 1],
                    )

                return [p1, p2]

            def qproj_parts(pt, ch):
                cell = {}
                half, c2 = ch // 2, ch % 2

                def p1():
                    cell["pp"] = psum.tile([128, 512], F32, tag="C", bufs=2, name="pq")
                    for kk in range(4):
                        nc.tensor.matmul(
                            cell["pp"],
                            lhsT=wq_sb[:, kk, 128 * pt : 128 * (pt + 1)],
                            rhs=qt_t[half][kk][:, 512 * c2 : 512 * (c2 + 1)],
                            start=(kk == 0),
                            stop=False,
                        )

                def p2():
                    for kk in range(4, KT):
                        nc.tensor.matmul(
                            cell["pp"],
                            lhsT=wq_sb[:, kk, 128 * pt : 128 * (pt + 1)],
                            rhs=qt_t[half][kk][:, 512 * c2 : 512 * (c2 + 1)],
                            start=False,
                            stop=(kk == KT - 1),
                        )
                    nc.vector.tensor_scalar_add(
                        qhT[:, pt, 512 * ch : 512 * (ch + 1)],
                        cell["pp"],
                        bq_sb[:, pt : pt + 1],
                    )

                return [p1, p2]

            def vproj_parts(sb):
                cell = {}
                half, s2 = sb // 8, sb % 8

                def p1():
                    cell["pv"] = psum.tile([128, 512], F32, tag="C", bufs=2, name="pv")
                    for kk in range(4):
                        nc.tensor.matmul(
                            cell["pv"][:, 0:PL],
                            lhsT=vt_t[half][kk][:, 128 * s2 : 128 * (s2 + 1)],
                            rhs=wv_sb[:, kk, :],
                            start=(kk == 0),
                            stop=False,
                        )

                def p2():
                    pv = cell["pv"]
                    for kk in range(4, KT):
                        nc.tensor.matmul(
                            pv[:, 0:PL],
                            lhsT=vt_t[half][kk][:, 128 * s2 : 128 * (s2 + 1)],
                            rhs=wv_sb[:, kk, :],
                            start=False,
                            stop=(kk == KT - 1),
                        )
                    # pv col order is [h0,h2 | h1,h3] (host-permuted wv), so
                    # both copy destinations are affine APs of vh[:, sb]
                    dst = vh[:, sb, :, :].rearrange("p (a b) e -> p a b e", a=2)
                    pvv = pv[:, 0:PL].rearrange("p (g a e) -> p g a e", g=2, a=2)
                    bvv = bv_bc.rearrange("p (g a e) -> p g a e", g=2, a=2)
                    nc.vector.tensor_tensor(
                        out=dst[:, :, 0, 0:64],
                        in0=pvv[:, 0, :, :],
                        in1=bvv[:, 0, :, :],
                        op=ADD,
                    )
                    nc.vector.tensor_tensor(
                        out=dst[:, :, 1, 64:128],
                        in0=pvv[:, 1, :, :],
                        in1=bvv[:, 1, :, :],
                        op=ADD,
                    )

                return [p1, p2]

            def outproj_parts(ib):
                cell = {}

                def part(oc):
                    def p():
                        if oc == 0:
                            cell["ob"] = obp.tile([128, D], DT, tag="ob", name="ob")
                        po = psum.tile([128, 512], F32, tag="C", bufs=2, name="po")
                        for t in range(2):
                            nc.tensor.matmul(
                                po,
                                lhsT=xn2[:, t, 128 * ib : 128 * (ib + 1)],
                                rhs=wf_sb[:, t, 512 * oc : 512 * (oc + 1)],
                                start=(t == 0),
                                stop=(t == 1),
                            )
                        nc.vector.tensor_copy(
                            out=cell["ob"][:, 512 * oc : 512 * (oc + 1)], in_=po
                        )
                        if oc == 1:
                            nc.sync.dma_start(
                                out_d.ap()[128 * ib : 128 * (ib + 1), :], cell["ob"]
                            )

                    return p

                return [part(0), part(1)]

            # fill order is deadline-driven: part k is consumed at pump k+1;
            # pumps run 1/score-block (+1 per chunk end), so e.g. qproj(0,1)
            # must sit at idx<=3 to precede chunk (0,1)'s scores
            fill = []
            fill.extend(qproj_parts(0, 1))          # idx 0-1
            for sb in range(4):
                fill.extend(vproj_parts(sb))        # 2-9
            fill.extend(qproj_parts(0, 2))          # 10-11
            for sb in range(4, 8):
                fill.extend(vproj_parts(sb))        # 12-19
            fill.extend(qproj_parts(0, 3))          # 20-21
            for sb in range(8, 12):
                fill.extend(vproj_parts(sb))        # 22-29
            for ch in range(CH):
                fill.extend(kproj_parts(1, ch))     # 30-37
            for sb in range(12, 16):
                fill.extend(vproj_parts(sb))        # 38-45
            fill.extend(qproj_parts(1, 3))          # 46-47
            fill.extend(qproj_parts(1, 2))          # 48-49
            fill.extend(qproj_parts(1, 1))          # 50-51
            fill.extend(qproj_parts(1, 0))          # 52-53
            # minimum fill progress required before each chunk's scores
            min_idx = {(0, 1): 2, (0, 2): 12, (0, 3): 22,
                       (1, 3): 48, (1, 2): 50, (1, 1): 52, (1, 0): 54}

            fpos = [0]

            def pump(n=1):
                while n > 0 and fpos[0] < len(fill):
                    fill[fpos[0]]()
                    fpos[0] += 1
                    n -= 1

            # ---- preamble: khT pair 0, qhT (0, c0) ----
            for ch in range(CH):
                for p in kproj_parts(0, ch):
                    p()
                if ch == 0:
                    issue_v_dmas()
            for p in qproj_parts(0, 0):
                p()

            # ---- attention ----
            def score_block(hb, c, bj, band):
                i0 = 128 * bj if band else 512 * c
                w = 512 * (c + 1) - i0
                psp = psum.tile([128, 2, 512], F32, tag="A", bufs=2, name="psp")
                for par in range(2):
                    nc.tensor.matmul(
                        psp[:, par, 0:w],
                        lhsT=khT[
                            64 * par : 64 * par + 64, hb, 128 * bj : 128 * (bj + 1)
                        ],
                        rhs=qhT[64 * par : 64 * par + 64, hb, i0 : i0 + w],
                        start=True,
                        stop=True,
                    )
                et = epool.tile([128, 2, 512], DT, tag="et", name="et")
                nc.scalar.activation(et[:, :, 0:w], psp[:, :, 0:w], EXP)
                if band:
                    # zero the masked (strictly-upper) part of the diagonal
                    # 128-block, always the first 128 columns of the slice
                    nc.vector.tensor_tensor(
                        out=et[:, :, 0:128], in0=et[:, :, 0:128], in1=tri_sb, op=MUL
                    )
                return et

            def av_gen(hb, c, ets, px):
                nbj = 4 * c + 4
                for bj in range(nbj):
                    band = bj >= 4 * c
                    i0 = 128 * bj if band else 512 * c
                    w = 512 * (c + 1) - i0
                    o = i0 - 512 * c
                    for par in range(2):
                        nc.tensor.matmul(
                            px[:, par, o : o + w],
                            lhsT=vh[:, bj, 2 * hb + par, :],
                            rhs=ets[bj][:, par, 0:w],
                            start=(bj == 0),
                            stop=(bj == nbj - 1),
                        )
                    yield

            def normalize(hb, c, px):
                # sums rows live at different partitions per par (96 / 32),
                # same free range, so one [128, 512] tile serves both
                # custom DVE ops need SBUF input (bit-trick seed misreads
                # PSUM on HW): stage the sums rows first
                sst = nrm.tile([128, 512], F32, tag="sst", name="sst")
                sraw = nrm.tile([128, 512], F32, tag="sraw", name="sraw")
                nc.vector.tensor_copy(
                    out=sst[ONE_EVEN : ONE_EVEN + 1, :],
                    in_=px[ONE_EVEN : ONE_EVEN + 1, 0, :],
                )
                nc.vector.tensor_copy(
                    out=sst[ONE_ODD : ONE_ODD + 1, :],
                    in_=px[ONE_ODD : ONE_ODD + 1, 1, :],
                )
                nc.vector.reciprocal(
                    sraw[ONE_EVEN : ONE_EVEN + 1, :],
                    sst[ONE_EVEN : ONE_EVEN + 1, :],
                )
                nc.vector.reciprocal(
                    sraw[ONE_ODD : ONE_ODD + 1, :],
                    sst[ONE_ODD : ONE_ODD + 1, :],
                )
                nc.vector.tensor_copy(
                    out=sums_bf[ONE_EVEN : ONE_EVEN + 1, :],
                    in_=sraw[ONE_EVEN : ONE_EVEN + 1, :],
                )
                nc.vector.tensor_copy(
                    out=sums_bf[ONE_ODD : ONE_ODD + 1, :],
                    in_=sraw[ONE_ODD : ONE_ODD + 1, :],
                )
                rbc = psum.tile([128, 512], F32, tag="C", bufs=2, name="rbc")
                nc.tensor.matmul(
                    rbc,
                    lhsT=sel2_sb,
                    rhs=sums_bf,
                    start=True,
                    stop=True,
                )
                # DVE can read only one PSUM operand per op: stage rbc in SBUF
                rbs = nrm.tile([128, 512], F32, tag="rbs", bufs=1, name="rbs")
                nc.vector.tensor_copy(out=rbs, in_=rbc)
                cc = slice(512 * c, 512 * (c + 1))
                nc.vector.tensor_tensor(
                    out=xn2[0:64, hb, cc], in0=px[0:64, 0, :], in1=rbs[0:64, :], op=MUL
                )
                nc.vector.tensor_tensor(
                    out=xn2[64:128, hb, cc],
                    in0=px[64:128, 1, :],
                    in1=rbs[64:128, :],
                    op=MUL,
                )

            seq = [(0, 0), (0, 1), (0, 2), (0, 3), (1, 3), (1, 2), (1, 1), (1, 0)]
            prev = None  # (hb, c, av generator, px)
            for hb, c in seq:
                nbj = 4 * c + 4
                nbj_prev = 4 * prev[1] + 4 if prev else 0
                drain_start = max(0, nbj - nbj_prev)
                pump(max(0, min_idx.get((hb, c), 0) - fpos[0]))
                def finish_prev():
                    normalize(prev[0], prev[1], prev[3])
                    if prev[0] == 1:
                        for ib in range(4 * prev[1], 4 * prev[1] + 4):
                            fill.extend(outproj_parts(ib))

                prev_done = prev is None
                ets = []
                for bj in range(nbj):
                    ets.append(score_block(hb, c, bj, bj >= 4 * c))
                    pump(1)
                    if prev is not None and bj >= drain_start:
                        a = next(prev[2], "end")
                        b = next(prev[2], "end")
                        if b == "end" and not prev_done:
                            finish_prev()
                            prev_done = True
                if not prev_done:
                    for _ in prev[2]:
                        pass
                    finish_prev()
                    pump(1)
                px = psum.tile([128, 2, 512], F32, tag="B", bufs=1, name="px")
                prev = (hb, c, av_gen(hb, c, ets, px), px)

            for _ in prev[2]:
                pass
            normalize(prev[0], prev[1], prev[3])
            for ib in range(4 * prev[1], 4 * prev[1] + 4):
                fill.extend(outproj_parts(ib))
            pump(len(fill))
            assert dummy_gate[0]

    nc.compile()
    _cache["nc"] = nc
    return nc


def _wlayout(wT):
    # [D, PL] -> SBUF layout [128, KT, PL]
    return np.ascontiguousarray(wT.reshape(KT, 128, PL).transpose(1, 0, 2)).astype(NP_DT)


def _flayout(wT):
    # [PL, D] -> SBUF layout [128, 2, D]
    return np.ascontiguousarray(wT.reshape(2, 128, D).transpose(1, 0, 2)).astype(NP_DT)


def _vperm():
    # pv/wv column order [h0, h2, h1, h3] so both V-copy destination APs
    # (even heads -> slots 0:64, odd heads -> slots 64:128) are affine
    idx = []
    for h in (0, 2, 1, 3):
        idx.extend(range(h * HD, (h + 1) * HD))
    return np.array(idx)


def make_in_maps(q, k, v, mask, Wq, bq, Wk, bk, Wv, bv, Wf, bf):
    scale = 1.0 / np.sqrt(np.float32(HD))
    f32 = np.float32
    m = np.asarray(mask[0, 0])
    # multiplicative causal mask for the diagonal 128-block, transposed
    # (scores are [key, query]), duplicated for both pair members
    t1 = (m[:128, :128].T != 0).astype(f32)
    tri2 = np.ascontiguousarray(np.broadcast_to(t1[:, None, :], (128, 2, 128))).astype(
        NP_DT
    )
    vperm = _vperm()
    in_maps = []
    for c in range(8):
        b, g = c // 4, c % 4
        sl = slice(g * PL, (g + 1) * PL)
        wv_l = np.asarray(Wv)[sl, :].T[:, vperm]  # [D, PL] col-permuted
        bv_l = np.asarray(bv)[sl][vperm]
        in_maps.append(
            {
                "qt": np.ascontiguousarray((np.asarray(q[b]).T * scale)).astype(NP_DT),
                "kt": np.ascontiguousarray(np.asarray(k[b]).T).astype(NP_DT),
                "vt": np.ascontiguousarray(np.asarray(v[b]).T).astype(NP_DT),
                "wq": _wlayout(np.asarray(Wq)[sl, :].T),
                "wk": _wlayout(np.asarray(Wk)[sl, :].T),
                "wv": _wlayout(wv_l),
                "wf": _flayout(np.asarray(Wf)[:, sl].T),
                "bq2": np.ascontiguousarray(
                    (np.asarray(bq)[sl] * scale).astype(f32).reshape(2, 128).T
                ),
                "bk2": np.ascontiguousarray(
                    np.asarray(bk)[sl].astype(f32).reshape(2, 128).T
                ),
                "bv1": bv_l.astype(f32).reshape(1, PL),
                "tri2": tri2,
            }
        )
    return in_maps


def _mask_is_causal(mask):
    m = np.asarray(mask[0, 0])
    return bool(np.array_equal(m != 0, np.tril(np.ones((S, S), bool))))


def _numpy_fallback(q, k, v, mask, Wq, bq, Wk, bk, Wv, bv, Wf, bf):
    out = np.empty((B, S, D), np.float32)
    m = np.asarray(mask[0, 0])
    for b in range(B):
        qh = (np.asarray(q[b]) @ np.asarray(Wq).T + bq).reshape(S, H, HD)
        kh = (np.asarray(k[b]) @ np.asarray(Wk).T + bk).reshape(S, H, HD)
        vhh = (np.asarray(v[b]) @ np.asarray(Wv).T + bv).reshape(S, H, HD)
        x = np.empty((S, H, HD), np.float32)
        for hh in range(H):
            sc = qh[:, hh] @ kh[:, hh].T / np.sqrt(np.float32(HD))
            sc = np.where(m == 0, np.float32(-1e9), sc)
            sc = sc - sc.max(-1, keepdims=True)
            e = np.exp(sc)
            x[:, hh] = (e / e.sum(-1, keepdims=True)) @ vhh[:, hh]
        out[b] = x.reshape(S, D) @ np.asarray(Wf).T + bf
    return out


def kernel(q, k, v, mask, Wq, bq, Wk, bk, Wv, bv, Wf, bf):
    global last_results
    if not _mask_is_causal(mask):
        return _numpy_fallback(q, k, v, mask, Wq, bq, Wk, bk, Wv, bv, Wf, bf)
    nc = build_program()
    in_maps = make_in_maps(q, k, v, mask, Wq, bq, Wk, bk, Wv, bv, Wf, bf)
    res = run_bass_kernel_spmd(nc, in_maps, core_ids=list(range(8)))
    last_results = res
    out = np.zeros((B, S, D), np.float32)
    for c in range(8):
        out[c // 4] += res.results[c]["out"].astype(np.float32)
    out += np.asarray(bf, np.float32)[None, None, :]
    return out


# revision 16
# speedup vs baseline: 1.2868x; 1.2868x over previous
"""Multi-head attention (B=2, S=2048, D=1024, H=16) on 8 TRN2 NeuronCores.

Sharding: batch x head-group. Core c handles batch b=c//4 and heads
[4g, 4g+4) with g=c%4 (column-parallel QKV projections, row-parallel
output projection). Each core emits a partial [S, D] output; the host
sums the 4 partials per batch and adds bf (the row-parallel reduce).

v2 schedule: the attention inner loop is ScalarE-bound (exp), so all
projection / output-projection matmuls are interleaved into the
attention emission stream in ~0.9us sub-quanta to keep the PE
continuously busy (full HAM clock). Causal masking is a multiplicative
bf16 0/1 tile applied post-exp on GpSimd (SBUF-only engine, otherwise
idle). Softmax renormalization per (head-pair, chunk): row sums fall
out of the AV matmul via a ones column in vh (even local heads put
values in px partitions 0:64 with the ones column at slot 96; odd heads
at 64:128 with ones at slot 32, so normalized outputs land directly in
both halves of xn2 without any partition-crossing DMA),
reciprocal_approx_fast on the sums row, then K=1 ones-vector matmuls
broadcast 1/s across partitions. AV weights are zero-padded to 128
columns so fast-weight-load stays enabled.
"""

import numpy as np
import ml_dtypes

import concourse.bass as bass
import concourse.tile as tile
from concourse import bacc, mybir
from concourse.bass_utils import run_bass_kernel_spmd

B, S, D, H = 2, 2048, 1024, 16
HD = D // H          # 64
HL = H // 4          # 4 heads per core
PL = HL * HD         # 256 local projection dim
KT = D // 128        # 8 contraction blocks
SB = S // 128        # 16 sequence blocks of 128
CH = S // 512        # 4 sequence chunks of 512
F32 = mybir.dt.float32
DT = mybir.dt.bfloat16
NP_DT = ml_dtypes.bfloat16

_cache = {}
last_results = None

ADD = mybir.AluOpType.add
MUL = mybir.AluOpType.mult
EXP = mybir.ActivationFunctionType.Exp

# ones-column slot in vh per local-head parity (= sums partition in px);
# both 32-aligned so the K=1 broadcast matmuls get legal tile_positions
ONE_EVEN = 64
ONE_ODD = 32


def build_program():
    if "nc" in _cache:
        return _cache["nc"]
    nc = bacc.Bacc("TRN2", target_bir_lowering=False, debug=False, num_devices=8)

    qt_d = nc.dram_tensor("qt", [D, S], DT, kind="ExternalInput")
    kt_d = nc.dram_tensor("kt", [D, S], DT, kind="ExternalInput")
    vt_d = nc.dram_tensor("vt", [D, S], DT, kind="ExternalInput")
    wq_d = nc.dram_tensor("wq", [128, KT, PL], DT, kind="ExternalInput")
    wk_d = nc.dram_tensor("wk", [128, KT, PL], DT, kind="ExternalInput")
    wv_d = nc.dram_tensor("wv", [128, KT, PL], DT, kind="ExternalInput")
    wf_d = nc.dram_tensor("wf", [128, 2, D], DT, kind="ExternalInput")
    bq_d = nc.dram_tensor("bq2", [128, 2], F32, kind="ExternalInput")
    bk_d = nc.dram_tensor("bk2", [128, 2], F32, kind="ExternalInput")
    bv_d = nc.dram_tensor("bv1", [1, PL], F32, kind="ExternalInput")
    tri_d = nc.dram_tensor("tri2", [128, 2, 128], DT, kind="ExternalInput")
    scr_d = nc.dram_tensor("scr", [1, 512], DT, kind="ExternalOutput")
    out_d = nc.dram_tensor("out", [S, D], DT, kind="ExternalOutput")

    with tile.TileContext(nc) as tc:
        with (
            tc.tile_pool(name="singles", bufs=1) as singles,
            tc.tile_pool(name="kp", bufs=8) as kp,
            tc.tile_pool(name="qp", bufs=16) as qp,
            tc.tile_pool(name="vp", bufs=16) as vp,
            tc.tile_pool(name="epool", bufs=17) as epool,
            tc.tile_pool(name="nrm", bufs=2) as nrm,
            tc.tile_pool(name="obp", bufs=3) as obp,
            tc.tile_pool(name="psum", bufs=2, space="PSUM") as psum,
        ):
            wq_sb = singles.tile([128, KT, PL], DT)
            wk_sb = singles.tile([128, KT, PL], DT)
            wv_sb = singles.tile([128, KT, PL], DT)
            wf_sb = singles.tile([128, 2, D], DT)
            bq_sb = singles.tile([128, 2], F32)
            bk_sb = singles.tile([128, 2], F32)
            bv_bc = singles.tile([128, PL], F32)
            tri_sb = singles.tile([128, 2, 128], DT)
            sel2_sb = singles.tile([128, 128], DT)
            sums_bf = singles.tile([128, 512], DT)

            khT = singles.tile([128, 2, S], DT)   # [par*64+d, pair, s]
            qhT = singles.tile([128, 2, S], DT)
            # vh: [s within blk, sblk, local h, slot]; even h: vals 0:64,
            # ones@96; odd h: vals 64:128, ones@32; zeros elsewhere (full
            # 128-wide AV weights keep FWL enabled)
            vh = singles.tile([128, SB, HL, 128], DT)
            xn2 = singles.tile([128, 2, S], DT)   # normalized attn out

            nc.vector.memset(sel2_sb, 0.0)
            nc.vector.memset(sel2_sb[ONE_EVEN : ONE_EVEN + 1, 0:64], 1.0)
            nc.vector.memset(sel2_sb[ONE_ODD : ONE_ODD + 1, 64:128], 1.0)
            nc.vector.memset(sums_bf, 0.0)
            for h in range(HL):
                if h % 2 == 0:
                    nc.vector.memset(vh[:, :, h, 64:65], 1.0)
                    nc.vector.memset(vh[:, :, h, 65:128], 0.0)
                else:
                    nc.vector.memset(vh[:, :, h, 0:32], 0.0)
                    nc.vector.memset(vh[:, :, h, 32:33], 1.0)
                    nc.vector.memset(vh[:, :, h, 33:64], 0.0)

            # ---- DMA issues ----
            # all inputs on the sync queue: one logical FIFO whose issue
            # order is the HBM priority order (k -> q half0 -> v half0 ->
            # q half1 -> v half1)
            nc.sync.dma_start(wk_sb, wk_d.ap())
            nc.sync.dma_start(bk_sb, bk_d.ap())
            kt_t = []
            for kk in range(KT):
                t = kp.tile([128, S], DT, tag="k", name=f"kt{kk}")
                nc.sync.dma_start(t, kt_d.ap()[128 * kk : 128 * (kk + 1), :])
                kt_t.append(t)
            nc.sync.dma_start(wq_sb, wq_d.ap())
            nc.sync.dma_start(bq_sb, bq_d.ap())
            qt_t = [[None] * KT, [None] * KT]
            vt_t = [[None] * KT, [None] * KT]
            for kk in range(KT):
                t = qp.tile([128, 1024], DT, tag="q", name=f"q0_{kk}")
                nc.sync.dma_start(
                    t, qt_d.ap()[128 * kk : 128 * (kk + 1), 0:1024]
                )
                qt_t[0][kk] = t
            nc.sync.dma_start(wv_sb, wv_d.ap())
            nc.sync.dma_start(bv_bc, bv_d.ap().to_broadcast([128, PL]))
            nc.sync.dma_start(tri_sb, tri_d.ap())
            for kk in range(KT):
                t = vp.tile([128, 1024], DT, tag="v", name=f"v0_{kk}")
                nc.sync.dma_start(t, vt_d.ap()[128 * kk : 128 * (kk + 1), 0:1024])
                vt_t[0][kk] = t
            for kk in range(KT):
                t = qp.tile([128, 1024], DT, tag="q", name=f"q1_{kk}")
                nc.sync.dma_start(
                    t, qt_d.ap()[128 * kk : 128 * (kk + 1), 1024:2048]
                )
                qt_t[1][kk] = t
            for kk in range(KT):
                t = vp.tile([128, 1024], DT, tag="v", name=f"v1_{kk}")
                nc.sync.dma_start(
                    t, vt_d.ap()[128 * kk : 128 * (kk + 1), 1024:2048]
                )
                vt_t[1][kk] = t
            nc.sync.dma_start(wf_sb, wf_d.ap())

            # ---- PE work quanta (two ~0.9us parts each) ----
            def kproj_parts(pt, ch, pre=False):
                cell = {}

                def p1():
                    if pre:
                        base = psum.tile(
                            [128, 2, 512], F32, tag="A", bufs=2, name="ppa"
                        )
                        cell["pp"] = base[:, 0, :]
                    else:
                        cell["pp"] = psum.tile(
                            [128, 512], F32, tag="C", bufs=2, name="pp"
                        )
                    for kk in range(4):
                        nc.tensor.matmul(
                            cell["pp"],
                            lhsT=wk_sb[:, kk, 128 * pt : 128 * (pt + 1)],
                            rhs=kt_t[kk][:, 512 * ch : 512 * (ch + 1)],
                            start=(kk == 0),
                            stop=False,
                        )

                def p2():
                    for kk in range(4, KT):
                        nc.tensor.matmul(
                            cell["pp"],
                            lhsT=wk_sb[:, kk, 128 * pt : 128 * (pt + 1)],
                            rhs=kt_t[kk][:, 512 * ch : 512 * (ch + 1)],
                            start=False,
                            stop=(kk == KT - 1),
                        )
                    nc.vector.tensor_scalar_add(
                        khT[:, pt, 512 * ch : 512 * (ch + 1)],
                        cell["pp"],
                        bk_sb[:, pt : pt + 1],
                    )

                return [p1, p2]

            def qproj_parts(pt, ch):
                cell = {}
                half, c2 = ch // 2, ch % 2

                def p1():
                    cell["pp"] = psum.tile([128, 512], F32, tag="C", bufs=2, name="pq")
                    for kk in range(4):
                        nc.tensor.matmul(
                            cell["pp"],
                            lhsT=wq_sb[:, kk, 128 * pt : 128 * (pt + 1)],
                            rhs=qt_t[half][kk][:, 512 * c2 : 512 * (c2 + 1)],
                            start=(kk == 0),
                            stop=False,
                        )

                def p2():
                    for kk in range(4, KT):
                        nc.tensor.matmul(
                            cell["pp"],
                            lhsT=wq_sb[:, kk, 128 * pt : 128 * (pt + 1)],
                            rhs=qt_t[half][kk][:, 512 * c2 : 512 * (c2 + 1)],
                            start=False,
                            stop=(kk == KT - 1),
                        )
                    nc.vector.tensor_scalar_add(
                        qhT[:, pt, 512 * ch : 512 * (ch + 1)],
                        cell["pp"],
                        bq_sb[:, pt : pt + 1],
                    )

                return [p1, p2]

            def vproj_parts(sb):
                cell = {}
                half, s2 = sb // 8, sb % 8

                def p1():
                    cell["pv"] = psum.tile([128, 512], F32, tag="C", bufs=2, name="pv")
                    for kk in range(4):
                        nc.tensor.matmul(
                            cell["pv"][:, 0:PL],
                            lhsT=vt_t[half][kk][:, 128 * s2 : 128 * (s2 + 1)],
                            rhs=wv_sb[:, kk, :],
                            start=(kk == 0),
                            stop=False,
                        )

                def p2():
                    pv = cell["pv"]
                    for kk in range(4, KT):
                        nc.tensor.matmul(
                            pv[:, 0:PL],
                            lhsT=vt_t[half][kk][:, 128 * s2 : 128 * (s2 + 1)],
                            rhs=wv_sb[:, kk, :],
                            start=False,
                            stop=(kk == KT - 1),
                        )
                    # pv col order is [h0,h2 | h1,h3] (host-permuted wv), so
                    # both copy destinations are affine APs of vh[:, sb]
                    dst = vh[:, sb, :, :].rearrange("p (a b) e -> p a b e", a=2)
                    pvv = pv[:, 0:PL].rearrange("p (g a e) -> p g a e", g=2, a=2)
                    bvv = bv_bc.rearrange("p (g a e) -> p g a e", g=2, a=2)
                    nc.vector.tensor_tensor(
                        out=dst[:, :, 0, 0:64],
                        in0=pvv[:, 0, :, :],
                        in1=bvv[:, 0, :, :],
                        op=ADD,
                    )
                    nc.vector.tensor_tensor(
                        out=dst[:, :, 1, 64:128],
                        in0=pvv[:, 1, :, :],
                        in1=bvv[:, 1, :, :],
                        op=ADD,
                    )

                return [p1, p2]

            def outproj_parts(ib):
                cell = {}

                def part(oc):
                    def p():
                        if oc == 0:
                            cell["ob"] = obp.tile([128, D], DT, tag="ob", name="ob")
                        po = psum.tile([128, 512], F32, tag="C", bufs=2, name="po")
                        for t in range(2):
                            nc.tensor.matmul(
                                po,
                                lhsT=xn2[:, t, 128 * ib : 128 * (ib + 1)],
                                rhs=wf_sb[:, t, 512 * oc : 512 * (oc + 1)],
                                start=(t == 0),
                                stop=(t == 1),
                            )
                        nc.vector.tensor_copy(
                            out=cell["ob"][:, 512 * oc : 512 * (oc + 1)], in_=po
                        )
                        if oc == 1:
                            nc.sync.dma_start(
                                out_d.ap()[128 * ib : 128 * (ib + 1), :], cell["ob"]
                            )

                    return p

                return [part(0), part(1)]

            # fill order is deadline-driven: part k is consumed at pump k+1;
            # pumps run 1/score-block (+1 per chunk end), so e.g. qproj(0,1)
            # must sit at idx<=3 to precede chunk (0,1)'s scores
            fill = []
            fill.extend(qproj_parts(0, 1))          # idx 0-1
            for sb in range(4):
                fill.extend(vproj_parts(sb))        # 2-9
            fill.extend(qproj_parts(0, 2))          # 10-11
            for sb in range(4, 8):
                fill.extend(vproj_parts(sb))        # 12-19
            fill.extend(qproj_parts(0, 3))          # 20-21
            for sb in range(8, 12):
                fill.extend(vproj_parts(sb))        # 22-29
            for ch in range(CH):
                fill.extend(kproj_parts(1, ch))     # 30-37
            for sb in range(12, 16):
                fill.extend(vproj_parts(sb))        # 38-45
            fill.extend(qproj_parts(1, 3))          # 46-47
            fill.extend(qproj_parts(1, 2))          # 48-49
            fill.extend(qproj_parts(1, 1))          # 50-51
            fill.extend(qproj_parts(1, 0))          # 52-53
            # minimum fill progress required before each chunk's scores
            min_idx = {(0, 1): 2, (0, 2): 12, (0, 3): 22,
                       (1, 3): 48, (1, 2): 50, (1, 1): 52, (1, 0): 54}

            fpos = [0]

            def pump(n=1):
                while n > 0 and fpos[0] < len(fill):
                    fill[fpos[0]]()
                    fpos[0] += 1
                    n -= 1

            # ---- preamble: khT pair 0, qhT (0, c0) ----
            for ch in range(CH):
                for p in kproj_parts(0, ch, pre=(ch % 2 == 1)):
                    p()
            for p in qproj_parts(0, 0):
                p()

            # ---- attention ----
            def score_block(hb, c, bj, band):
                i0 = 128 * bj if band else 512 * c
                w = 512 * (c + 1) - i0
                psp = psum.tile([128, 2, 512], F32, tag="A", bufs=2, name="psp")
                for par in range(2):
                    nc.tensor.matmul(
                        psp[:, par, 0:w],
                        lhsT=khT[
                            64 * par : 64 * par + 64, hb, 128 * bj : 128 * (bj + 1)
                        ],
                        rhs=qhT[64 * par : 64 * par + 64, hb, i0 : i0 + w],
                        start=True,
                        stop=True,
                    )
                et = epool.tile([128, 2, 512], DT, tag="et", name="et")
                nc.scalar.activation(et[:, :, 0:w], psp[:, :, 0:w], EXP)
                if band:
                    # zero the masked (strictly-upper) part of the diagonal
                    # 128-block, always the first 128 columns of the slice
                    nc.vector.tensor_tensor(
                        out=et[:, :, 0:128], in0=et[:, :, 0:128], in1=tri_sb, op=MUL
                    )
                return et

            def av_gen(hb, c, ets, px):
                nbj = 4 * c + 4
                for bj in range(nbj):
                    band = bj >= 4 * c
                    i0 = 128 * bj if band else 512 * c
                    w = 512 * (c + 1) - i0
                    o = i0 - 512 * c
                    for par in range(2):
                        nc.tensor.matmul(
                            px[:, par, o : o + w],
                            lhsT=vh[:, bj, 2 * hb + par, :],
                            rhs=ets[bj][:, par, 0:w],
                            start=(bj == 0),
                            stop=(bj == nbj - 1),
                        )
                    yield

            def normalize(hb, c, px):
                # cast raw sums rows to bf16 (only ~1.3us of DVE before the
                # broadcast matmul so the in-order PE stream isn't stalled),
                # broadcast via the selector matmul, then take the
                # reciprocal on the PSUM->SBUF move
                nc.vector.tensor_copy(
                    out=sums_bf[ONE_EVEN : ONE_EVEN + 1, :],
                    in_=px[ONE_EVEN : ONE_EVEN + 1, 0, :],
                )
                nc.vector.tensor_copy(
                    out=sums_bf[ONE_ODD : ONE_ODD + 1, :],
                    in_=px[ONE_ODD : ONE_ODD + 1, 1, :],
                )
                rbc = psum.tile([128, 512], F32, tag="C", bufs=2, name="rbc")
                nc.tensor.matmul(
                    rbc,
                    lhsT=sel2_sb,
                    rhs=sums_bf,
                    start=True,
                    stop=True,
                )
                rbs = nrm.tile([128, 512], F32, tag="rbs", bufs=2, name="rbs")
                nc.vector.reciprocal(rbs, rbc)
                cc = slice(512 * c, 512 * (c + 1))
                nc.vector.tensor_tensor(
                    out=xn2[0:64, hb, cc], in0=px[0:64, 0, :], in1=rbs[0:64, :], op=MUL
                )
                nc.vector.tensor_tensor(
                    out=xn2[64:128, hb, cc],
                    in0=px[64:128, 1, :],
                    in1=rbs[64:128, :],
                    op=MUL,
                )

            seq = [(0, 0), (0, 1), (0, 2), (0, 3), (1, 3), (1, 2), (1, 1), (1, 0)]
            prev = None  # (hb, c, av generator, px)
            for hb, c in seq:
                nbj = 4 * c + 4
                nbj_prev = 4 * prev[1] + 4 if prev else 0
                drain_start = max(0, nbj - nbj_prev)
                pump(max(0, min_idx.get((hb, c), 0) - fpos[0]))
                def finish_prev():
                    normalize(prev[0], prev[1], prev[3])
                    if prev[0] == 1:
                        for ib in range(4 * prev[1], 4 * prev[1] + 4):
                            fill.extend(outproj_parts(ib))

                prev_done = prev is None
                ets = []
                for bj in range(nbj):
                    ets.append(score_block(hb, c, bj, bj >= 4 * c))
                    pump(1)
                    if prev is not None and bj >= drain_start:
                        a = next(prev[2], "end")
                        b = next(prev[2], "end")
                        if b == "end" and not prev_done:
                            finish_prev()
                            prev_done = True
                if not prev_done:
                    for _ in prev[2]:
                        pass
                    finish_prev()
                    pump(1)
                px = psum.tile([128, 2, 512], F32, tag="B", bufs=1, name="px")
                prev = (hb, c, av_gen(hb, c, ets, px), px)

            for _ in prev[2]:
                pass
            normalize(prev[0], prev[1], prev[3])
            for ib in range(4 * prev[1], 4 * prev[1] + 4):
                fill.extend(outproj_parts(ib))
            pump(len(fill))

    nc.compile()
    _cache["nc"] = nc
    return nc


def _wlayout(wT):
    # [D, PL] -> SBUF layout [128, KT, PL]
    return np.ascontiguousarray(wT.reshape(KT, 128, PL).transpose(1, 0, 2)).astype(NP_DT)


def _flayout(wT):
    # [PL, D] -> SBUF layout [128, 2, D]
    return np.ascontiguousarray(wT.reshape(2, 128, D).transpose(1, 0, 2)).astype(NP_DT)


def _vperm():
    # pv/wv column order [h0, h2, h1, h3] so both V-copy destination APs
    # (even heads -> slots 0:64, odd heads -> slots 64:128) are affine
    idx = []
    for h in (0, 2, 1, 3):
        idx.extend(range(h * HD, (h + 1) * HD))
    return np.array(idx)


def make_in_maps(q, k, v, mask, Wq, bq, Wk, bk, Wv, bv, Wf, bf):
    scale = 1.0 / np.sqrt(np.float32(HD))
    f32 = np.float32
    m = np.asarray(mask[0, 0])
    # multiplicative causal mask for the diagonal 128-block, transposed
    # (scores are [key, query]), duplicated for both pair members
    t1 = (m[:128, :128].T != 0).astype(f32)
    tri2 = np.ascontiguousarray(np.broadcast_to(t1[:, None, :], (128, 2, 128))).astype(
        NP_DT
    )
    vperm = _vperm()
    in_maps = []
    for c in range(8):
        b, g = c // 4, c % 4
        sl = slice(g * PL, (g + 1) * PL)
        wv_l = np.asarray(Wv)[sl, :].T[:, vperm]  # [D, PL] col-permuted
        bv_l = np.asarray(bv)[sl][vperm]
        in_maps.append(
            {
                "qt": np.ascontiguousarray((np.asarray(q[b]).T * scale)).astype(NP_DT),
                "kt": np.ascontiguousarray(np.asarray(k[b]).T).astype(NP_DT),
                "vt": np.ascontiguousarray(np.asarray(v[b]).T).astype(NP_DT),
                "wq": _wlayout(np.asarray(Wq)[sl, :].T),
                "wk": _wlayout(np.asarray(Wk)[sl, :].T),
                "wv": _wlayout(wv_l),
                "wf": _flayout(np.asarray(Wf)[:, sl].T),
                "bq2": np.ascontiguousarray(
                    (np.asarray(bq)[sl] * scale).astype(f32).reshape(2, 128).T
                ),
                "bk2": np.ascontiguousarray(
                    np.asarray(bk)[sl].astype(f32).reshape(2, 128).T
                ),
                "bv1": bv_l.astype(f32).reshape(1, PL),
                "tri2": tri2,
            }
        )
    return in_maps


def _mask_is_causal(mask):
    m = np.asarray(mask[0, 0])
    return bool(np.array_equal(m != 0, np.tril(np.ones((S, S), bool))))


def _numpy_fallback(q, k, v, mask, Wq, bq, Wk, bk, Wv, bv, Wf, bf):
    out = np.empty((B, S, D), np.float32)
    m = np.asarray(mask[0, 0])
    for b in range(B):
        qh = (np.asarray(q[b]) @ np.asarray(Wq).T + bq).reshape(S, H, HD)
        kh = (np.asarray(k[b]) @ np.asarray(Wk).T + bk).reshape(S, H, HD)
        vhh = (np.asarray(v[b]) @ np.asarray(Wv).T + bv).reshape(S, H, HD)
        x = np.empty((S, H, HD), np.float32)
        for hh in range(H):
            sc = qh[:, hh] @ kh[:, hh].T / np.sqrt(np.float32(HD))
            sc = np.where(m == 0, np.float32(-1e9), sc)
            sc = sc - sc.max(-1, keepdims=True)
            e = np.exp(sc)
            x[:, hh] = (e / e.sum(-1, keepdims=True)) @ vhh[:, hh]
        out[b] = x.reshape(S, D) @ np.asarray(Wf).T + bf
    return out


def kernel(q, k, v, mask, Wq, bq, Wk, bk, Wv, bv, Wf, bf):
    global last_results
    if not _mask_is_causal(mask):
        return _numpy_fallback(q, k, v, mask, Wq, bq, Wk, bk, Wv, bv, Wf, bf)
    nc = build_program()
    in_maps = make_in_maps(q, k, v, mask, Wq, bq, Wk, bk, Wv, bv, Wf, bf)
    res = run_bass_kernel_spmd(nc, in_maps, core_ids=list(range(8)))
    last_results = res
    out = np.zeros((B, S, D), np.float32)
    for c in range(8):
        out[c // 4] += res.results[c]["out"].astype(np.float32)
    out += np.asarray(bf, np.float32)[None, None, :]
    return out
